# revision 35
# baseline (speedup 1.0000x reference)
"""ButterflyLinear Trainium2 kernel.

Math: out[b, s, i] = (sum_o x[b, s, o] * W[o, i]) * mask[s, i], with
mask[s, i] = 1 iff 4s <= i < 4s+4 (stride-4 band). The band makes the
output block-diagonal: s-rows [128t, 128t+128) only touch output columns
[512t, 512t+512) -- an 8x compute reduction vs the full matmul.

Sharding (8 cores): core t owns s-block t for all 16 batches
(tensor-parallel split of W columns; no inter-core communication).

The kernel is input-stream-bound (HBM->SBUF ~360 GB/s per core), so the
design minimizes wire bytes and hides everything else under the stream:
  - x ships as float8 e3m4 (4 mantissa bits). The PE computes fp8
    products exactly into fp32 PSUM, and mixed-dtype matmul (fp16
    stationary x fp8 moving) is supported, both HW-verified. The only
    error is the host-side cast: band rel err 1.32e-2 vs the 2e-2 gate
    (W stays fp16: quantizing W too measures 1.81e-2 -- too close).
    Per core: 2.1MB fp8 x + 1.05MB fp16 W in, 0.5MB fp16 out.
  - W stationary: per (o-chunk c, s-sub-block h) one N=512 matmul
    streams all 16 batches (4 groups x 128 pack rows) -> 32 matmuls,
    one PSUM bank per h, accumulation chain over the 8 o-chunks.
  - 13 input DMAs on one HWDGE ring; issue order == arrival order ==
    matmul program order. x0 leads so the first matmul fires ~2us
    earlier; W pieces drop in just ahead of the chunks needing them and
    act as stream-slack for the PE (0.9us/chunk PE vs 0.72us/chunk fp8
    stream); the last chunk is h-halved so the final matmuls gate on
    small early completion sems (sems lag data by ~1.5-3us under load).
  - 9 dummy matmuls on a zeroed tile run during the initial DMA wait to
    trip the PE HAM clock-gate to 2.4GHz before real matmuls arrive,
    and a tiny ScalarE copy pre-triggers its ~1.5us ACT-table load.
  - Tail: c7/h01 lands -> banks 0,1 evac on Vector||Scalar (parallel
    PSUM reads on different banks) -> per-bank 128KB out DMAs split
    across the two HWDGE rings (sync/scalar); banks 2,3 chase.

Host extracts the 4-wide diagonal from the [n, (g, m)] blocks into the
zero-filled (16, 1024, 4096) result.
"""

import sys
from contextlib import ExitStack

import numpy as np

if "/opt/trn_rl_repo" not in sys.path:
    sys.path.insert(0, "/opt/trn_rl_repo")

import concourse.bass as bass  # noqa: E402,F401
import concourse.tile as tile  # noqa: E402
from concourse import bacc, mybir  # noqa: E402
from concourse.bass_utils import run_bass_kernel_spmd  # noqa: E402

B = 16  # batch
NT = 8  # s-blocks == cores
SB = 128  # s rows per block / pack rows per group
NC_ = 8  # o chunks
KC = 128  # o rows per chunk
NI = 512  # output columns per block
QB = 4  # batches packed per group
RW = SB // QB  # s-rows per sub-block (32)
NH = QB  # sub-blocks per s-block
NW = 4 * RW  # W window per sub-block (128)
NG = B // QB  # batch groups (4)

X_DT = mybir.dt.float8e3  # e3m4
W_DT = mybir.dt.float16
F32 = mybir.dt.float32
OUT_DT = mybir.dt.float16

_STATE: dict = {}


def _build():
    if "nc" in _STATE:
        return _STATE["nc"]

    nc = bacc.Bacc("TRN2", target_bir_lowering=False, debug=False, num_devices=NT)
    # xt[pair, p, cc, h, g, m] = x[4g + m//32, 128t + 32h + (m%32), 128*(2*pair+cc) + p]
    xt = nc.dram_tensor("xt", [4, KC, 2, NH, NG, SB], X_DT, kind="ExternalInput").ap()
    # wt[p, c, h, n] = W[128c + p, 512t + 128h + n]
    wt = nc.dram_tensor("wt", [KC, NC_, NH, NW], W_DT, kind="ExternalInput").ap()
    # out[h, n, (g, m)] = ps[h][n, 128g + m]
    out = nc.dram_tensor("out", [NH, NW, NG * SB], OUT_DT, kind="ExternalOutput").ap()

    with tile.TileContext(nc) as tc, ExitStack() as ctx:
        wp = ctx.enter_context(tc.tile_pool(name="w", bufs=1))
        xp = ctx.enter_context(tc.tile_pool(name="x", bufs=1))
        pp = ctx.enter_context(tc.tile_pool(name="ps", bufs=5, space="PSUM"))
        op = ctx.enter_context(tc.tile_pool(name="o", bufs=1))

        # HAM warm-up: dummy PE work with no input deps, sized to bridge
        # from kernel start (~cold 427ns/MM, warm 213ns) until the first
        # real matmul's data lands, so the clock-gate is at 2.4GHz and
        # never re-throttles (re-throttle fires after ~3.4us PE-idle).
        dm = op.tile([KC, NG * SB], X_DT, tag="dm")
        nc.gpsimd.memset(dm[:], 0)
        psd = pp.tile([NW, NG * SB], F32, tag="ps", name="ps_dummy")
        for _ in range(9):
            nc.tensor.matmul(psd[:], dm[:, 0:NW], dm[:], start=True, stop=True)
        # Touch ScalarE with a tiny copy so its ACT-table load (~1.5us)
        # happens now, not in front of the evacuation copies later.
        warm = op.tile([KC, 2], F32, tag="warm")
        nc.scalar.copy(warm[:], dm[:, 0:2])

        # One HWDGE ring, issue order == arrival order == matmul program
        # order. x0 leads (earliest possible first matmul); W arrives in
        # chunk-aligned pieces just ahead of the x chunks that need it.
        # The interleaved W bytes are slack for the PE: with x in fp8 the
        # PE (~0.9us per o-chunk) is slower than the x stream (~0.72us).
        xc = []
        x7ab = []

        def xdma(c):
            t = xp.tile([KC, NH, NG, SB], X_DT, tag=f"x{c}")
            nc.sync.dma_start(out=t[:], in_=xt[c // 2, :, c % 2])
            xc.append(t)

        def wdma(lo, hi, tag):
            t = wp.tile([KC, hi - lo, NH, NW], W_DT, tag=tag)
            nc.sync.dma_start(out=t[:], in_=wt[:, lo:hi])
            return t

        xdma(0)
        w01 = wdma(0, 2, "w01")
        xdma(1)
        xdma(2)
        w23 = wdma(2, 4, "w23")
        xdma(3)
        xdma(4)
        w4567 = wdma(4, 8, "w4567")
        xdma(5)
        xdma(6)
        for i in range(4):
            t = xp.tile([KC, 1, NG, SB], X_DT, tag=f"x7q{i}")
            nc.sync.dma_start(out=t[:], in_=xt[3, :, 1, i : i + 1])
            x7ab.append(t)

        ps = [pp.tile([NW, NG * SB], F32, tag="ps", name=f"ps_{h}") for h in range(NH)]

        def wslice(c, h):
            if c < 4:
                return (w01, w23)[c // 2][:, c % 2, h, :]
            return w4567[:, c - 4, h, :]

        def xmov(c, h):
            if c < 7:
                return xc[c][:, h]
            return x7ab[h][:, 0]

        for c in range(7):
            for h in range(NH):
                nc.tensor.matmul(
                    ps[h][:, :], wslice(c, h), xmov(c, h),
                    start=(c == 0), stop=False,
                )

        ot = [
            op.tile([NW, NG * SB], OUT_DT, tag=f"ot{h}", name=f"ot_{h}")
            for h in range(NH)
        ]
        # c7 h-quarters: bank h finishes as soon as its 65KB lands; each
        # bank's evac (Vector/Scalar alternating, different banks ->
        # parallel PSUM reads) and 128KB out DMA (sync/scalar rings
        # alternating) chase the stream tail bank by bank.
        # Vector evacs even banks, Scalar odd banks (different banks ->
        # parallel PSUM reads). Out-DMA issues go to sync (idle at the
        # tail) except the last, so Scalar's evac3 isn't stuck behind an
        # issue; out3 rides the scalar ring, overlapping sync's issues.
        for h in range(NH):
            nc.tensor.matmul(
                ps[h][:, :], wslice(7, h), xmov(7, h), start=False, stop=True,
            )
            if h % 2 == 0:
                nc.vector.tensor_copy(ot[h][:], ps[h][:])
            else:
                nc.scalar.copy(ot[h][:], ps[h][:])
            if h == 3:
                # Final bank: two 64KB halves on both rings so the last
                # receipts start earlier and overlap.
                nc.sync.dma_start(out=out[h, :, : NG * SB // 2], in_=ot[h][:, : NG * SB // 2])
                nc.scalar.dma_start(out=out[h, :, NG * SB // 2 :], in_=ot[h][:, NG * SB // 2 :])
            else:
                nc.sync.dma_start(out=out[h], in_=ot[h][:])

    nc.compile()
    _STATE["nc"] = nc
    return nc


def _shard(x, W):
    x = np.ascontiguousarray(np.asarray(x, dtype=np.float32)).astype(mybir.dt.np(X_DT))
    W = np.ascontiguousarray(np.asarray(W, dtype=np.float32)).astype(mybir.dt.np(W_DT))
    xr = x.reshape(NG, QB, NT, NH, RW, NC_, KC)  # [g, qi, t, h, r, c, p]
    xts = np.transpose(xr, (2, 5, 6, 3, 0, 1, 4)).reshape(NT, NC_, KC, NH, NG, SB)
    xts = xts.reshape(NT, 4, 2, KC, NH, NG, SB).transpose(0, 1, 3, 2, 4, 5, 6)
    # [t, pair, p, cc, h, g, m]
    wr = W.reshape(NC_, KC, NT, NH, NW)  # [c, p, t, h, n]
    wts = np.transpose(wr, (2, 1, 0, 3, 4))  # [t, p, c, h, n]
    return [
        {"xt": np.ascontiguousarray(xts[t]), "wt": np.ascontiguousarray(wts[t])}
        for t in range(NT)
    ]


def kernel(x, W, _trace=False, _trace_kwargs=None):
    nc = _build()
    in_maps = _shard(x, W)
    res = run_bass_kernel_spmd(
        nc,
        in_maps,
        list(range(NT)),
        trace=_trace,
        **(_trace_kwargs or {}),
    )
    _STATE["last_run"] = res
    band = np.empty((B, NT * SB, 4), dtype=np.float32)
    r_idx = np.arange(RW)
    for t in range(NT):
        blk4 = np.ascontiguousarray(
            res.results[t]["out"].astype(np.float32)
        )  # [h, n, 512]
        for h in range(NH):
            blk = blk4[h]  # [n=128, (g, m)=512]
            e = blk.strides[1]
            # value (g, qi, r, j) sits at blk[4r + j, 128g + 32qi + r]
            v = np.lib.stride_tricks.as_strided(
                blk,
                shape=(NG, QB, RW, 4),
                strides=(128 * e, 32 * e, blk.strides[0] * 4 + e, blk.strides[0]),
            )
            # [g, qi, r, j] -> b = 4g + qi, s = 128t + 32h + r
            band[:, 128 * t + 32 * h + r_idx, :] = v.reshape(B, RW, 4)
    s_idx = np.arange(NT * SB)
    y = np.zeros((B, NT * SB, NT * SB, 4), dtype=np.float32)
    y[:, s_idx, s_idx, :] = band
    return y.reshape(B, NT * SB, NT * NI)


# revision 37
# speedup vs baseline: 1.0202x; 1.0202x over previous
"""ButterflyLinear Trainium2 kernel.

Math: out[b, s, i] = (sum_o x[b, s, o] * W[o, i]) * mask[s, i], with
mask[s, i] = 1 iff 4s <= i < 4s+4 (stride-4 band). The band makes the
output block-diagonal: s-rows [128t, 128t+128) only touch output columns
[512t, 512t+512) -- an 8x compute reduction vs the full matmul.

Sharding (8 cores): core t owns s-block t for all 16 batches
(tensor-parallel split of W columns; no inter-core communication).

The kernel is input-stream-bound (HBM->SBUF ~360 GB/s per core), so the
design minimizes wire bytes and hides everything else under the stream:
  - x ships as float8 e3m4 (4 mantissa bits). The PE computes fp8
    products exactly into fp32 PSUM, and mixed-dtype matmul (fp16
    stationary x fp8 moving) is supported, both HW-verified. The only
    error is the host-side cast: band rel err 1.32e-2 vs the 2e-2 gate
    (W stays fp16: quantizing W too measures 1.81e-2 -- too close).
    Per core: 2.1MB fp8 x + 1.05MB fp16 W in, 0.5MB fp16 out.
  - W stationary: per (o-chunk c, s-sub-block h) one N=512 matmul
    streams all 16 batches (4 groups x 128 pack rows) -> 32 matmuls,
    one PSUM bank per h, accumulation chain over the 8 o-chunks.
  - 13 input DMAs on one HWDGE ring; issue order == arrival order ==
    matmul program order. x0 leads so the first matmul fires ~2us
    earlier; W pieces drop in just ahead of the chunks needing them and
    act as stream-slack for the PE (0.9us/chunk PE vs 0.72us/chunk fp8
    stream); the last chunk is h-halved so the final matmuls gate on
    small early completion sems (sems lag data by ~1.5-3us under load).
  - 9 dummy matmuls on a zeroed tile run during the initial DMA wait to
    trip the PE HAM clock-gate to 2.4GHz before real matmuls arrive,
    and a tiny ScalarE copy pre-triggers its ~1.5us ACT-table load.
  - Tail: c7/h01 lands -> banks 0,1 evac on Vector||Scalar (parallel
    PSUM reads on different banks) -> per-bank 128KB out DMAs split
    across the two HWDGE rings (sync/scalar); banks 2,3 chase.

Host extracts the 4-wide diagonal from the [n, (g, m)] blocks into the
zero-filled (16, 1024, 4096) result.
"""

import sys
from contextlib import ExitStack

import numpy as np

if "/opt/trn_rl_repo" not in sys.path:
    sys.path.insert(0, "/opt/trn_rl_repo")

import concourse.bass as bass  # noqa: E402,F401
import concourse.tile as tile  # noqa: E402
from concourse import bacc, mybir  # noqa: E402
from concourse.bass_utils import run_bass_kernel_spmd  # noqa: E402

B = 16  # batch
NT = 8  # s-blocks == cores
SB = 128  # s rows per block / pack rows per group
NC_ = 8  # o chunks
KC = 128  # o rows per chunk
NI = 512  # output columns per block
QB = 4  # batches packed per group
RW = SB // QB  # s-rows per sub-block (32)
NH = QB  # sub-blocks per s-block
NW = 4 * RW  # W window per sub-block (128)
NG = B // QB  # batch groups (4)

X_DT = mybir.dt.float8e3  # e3m4
W_DT = mybir.dt.float16
F32 = mybir.dt.float32
OUT_DT = mybir.dt.float16

_STATE: dict = {}


def _build():
    if "nc" in _STATE:
        return _STATE["nc"]

    nc = bacc.Bacc("TRN2", target_bir_lowering=False, debug=False, num_devices=NT)
    # xt[pair, p, cc, h, g, m] = x[4g + m//32, 128t + 32h + (m%32), 128*(2*pair+cc) + p]
    xt = nc.dram_tensor("xt", [4, KC, 2, NH, NG, SB], X_DT, kind="ExternalInput").ap()
    # wt[p, c, h, n] = W[128c + p, 512t + 128h + n]
    wt = nc.dram_tensor("wt", [KC, NC_, NH, NW], W_DT, kind="ExternalInput").ap()
    # out[h, n, (g, m)] = ps[h][n, 128g + m]
    out = nc.dram_tensor("out", [NH, NW, NG * SB], OUT_DT, kind="ExternalOutput").ap()

    with tile.TileContext(nc) as tc, ExitStack() as ctx:
        wp = ctx.enter_context(tc.tile_pool(name="w", bufs=1))
        xp = ctx.enter_context(tc.tile_pool(name="x", bufs=1))
        pp = ctx.enter_context(tc.tile_pool(name="ps", bufs=5, space="PSUM"))
        op = ctx.enter_context(tc.tile_pool(name="o", bufs=1))

        # HAM warm-up: dummy PE work with no input deps, sized to bridge
        # from kernel start (~cold 427ns/MM, warm 213ns) until the first
        # real matmul's data lands, so the clock-gate is at 2.4GHz and
        # never re-throttles (re-throttle fires after ~3.4us PE-idle).
        dm = op.tile([KC, NG * SB], X_DT, tag="dm")
        nc.gpsimd.memset(dm[:], 0)
        psd = pp.tile([NW, NG * SB], F32, tag="ps", name="ps_dummy")
        for _ in range(9):
            nc.tensor.matmul(psd[:], dm[:, 0:NW], dm[:], start=True, stop=True)
        # Touch ScalarE with a tiny copy so its ACT-table load (~1.5us)
        # happens now, not in front of the evacuation copies later.
        warm = op.tile([KC, 2], F32, tag="warm")
        nc.scalar.copy(warm[:], dm[:, 0:2])

        # One HWDGE ring, issue order == arrival order == matmul program
        # order. x0 leads (earliest possible first matmul); W arrives in
        # chunk-aligned pieces just ahead of the x chunks that need it.
        # The interleaved W bytes are slack for the PE: with x in fp8 the
        # PE (~0.9us per o-chunk) is slower than the x stream (~0.72us).
        xc = []
        x7ab = []

        def xdma(c):
            t = xp.tile([KC, NH, NG, SB], X_DT, tag=f"x{c}")
            nc.sync.dma_start(out=t[:], in_=xt[c // 2, :, c % 2])
            xc.append(t)

        def wdma(lo, hi, tag):
            t = wp.tile([KC, hi - lo, NH, NW], W_DT, tag=tag)
            nc.sync.dma_start(out=t[:], in_=wt[:, lo:hi])
            return t

        xdma(0)
        w01 = wdma(0, 2, "w01")
        xdma(1)
        xdma(2)
        w23 = wdma(2, 4, "w23")
        xdma(3)
        xdma(4)
        w45 = wdma(4, 6, "w45")
        xdma(5)
        w67 = wdma(6, 8, "w67")
        xdma(6)
        for i in range(4):
            t = xp.tile([KC, 1, NG, SB], X_DT, tag=f"x7q{i}")
            nc.sync.dma_start(out=t[:], in_=xt[3, :, 1, i : i + 1])
            x7ab.append(t)

        ps = [pp.tile([NW, NG * SB], F32, tag="ps", name=f"ps_{h}") for h in range(NH)]

        def wslice(c, h):
            return (w01, w23, w45, w67)[c // 2][:, c % 2, h, :]

        def xmov(c, h):
            if c < 7:
                return xc[c][:, h]
            return x7ab[h][:, 0]

        for c in range(7):
            for h in range(NH):
                nc.tensor.matmul(
                    ps[h][:, :], wslice(c, h), xmov(c, h),
                    start=(c == 0), stop=False,
                )

        ot = [
            op.tile([NW, NG * SB], OUT_DT, tag=f"ot{h}", name=f"ot_{h}")
            for h in range(NH)
        ]
        # c7 h-quarters: bank h finishes as soon as its 65KB lands; each
        # bank's evac (Vector/Scalar alternating, different banks ->
        # parallel PSUM reads) and 128KB out DMA (sync/scalar rings
        # alternating) chase the stream tail bank by bank.
        # Vector evacs even banks, Scalar odd banks (different banks ->
        # parallel PSUM reads). Out-DMA issues go to sync (idle at the
        # tail) except the last, so Scalar's evac3 isn't stuck behind an
        # issue; out3 rides the scalar ring, overlapping sync's issues.
        for h in range(NH):
            nc.tensor.matmul(
                ps[h][:, :], wslice(7, h), xmov(7, h), start=False, stop=True,
            )
            if h % 2 == 0:
                nc.vector.tensor_copy(ot[h][:], ps[h][:])
            else:
                nc.scalar.copy(ot[h][:], ps[h][:])
            if h == 3:
                # Final bank: two 64KB halves on both rings so the last
                # receipts start earlier and overlap.
                nc.sync.dma_start(out=out[h, :, : NG * SB // 2], in_=ot[h][:, : NG * SB // 2])
                nc.scalar.dma_start(out=out[h, :, NG * SB // 2 :], in_=ot[h][:, NG * SB // 2 :])
            else:
                nc.sync.dma_start(out=out[h], in_=ot[h][:])

    nc.compile()
    _STATE["nc"] = nc
    return nc


def _shard(x, W):
    x = np.ascontiguousarray(np.asarray(x, dtype=np.float32)).astype(mybir.dt.np(X_DT))
    W = np.ascontiguousarray(np.asarray(W, dtype=np.float32)).astype(mybir.dt.np(W_DT))
    xr = x.reshape(NG, QB, NT, NH, RW, NC_, KC)  # [g, qi, t, h, r, c, p]
    xts = np.transpose(xr, (2, 5, 6, 3, 0, 1, 4)).reshape(NT, NC_, KC, NH, NG, SB)
    xts = xts.reshape(NT, 4, 2, KC, NH, NG, SB).transpose(0, 1, 3, 2, 4, 5, 6)
    # [t, pair, p, cc, h, g, m]
    wr = W.reshape(NC_, KC, NT, NH, NW)  # [c, p, t, h, n]
    wts = np.transpose(wr, (2, 1, 0, 3, 4))  # [t, p, c, h, n]
    return [
        {"xt": np.ascontiguousarray(xts[t]), "wt": np.ascontiguousarray(wts[t])}
        for t in range(NT)
    ]


def kernel(x, W, _trace=False, _trace_kwargs=None):
    nc = _build()
    in_maps = _shard(x, W)
    res = run_bass_kernel_spmd(
        nc,
        in_maps,
        list(range(NT)),
        trace=_trace,
        **(_trace_kwargs or {}),
    )
    _STATE["last_run"] = res
    band = np.empty((B, NT * SB, 4), dtype=np.float32)
    r_idx = np.arange(RW)
    for t in range(NT):
        blk4 = np.ascontiguousarray(
            res.results[t]["out"].astype(np.float32)
        )  # [h, n, 512]
        for h in range(NH):
            blk = blk4[h]  # [n=128, (g, m)=512]
            e = blk.strides[1]
            # value (g, qi, r, j) sits at blk[4r + j, 128g + 32qi + r]
            v = np.lib.stride_tricks.as_strided(
                blk,
                shape=(NG, QB, RW, 4),
                strides=(128 * e, 32 * e, blk.strides[0] * 4 + e, blk.strides[0]),
            )
            # [g, qi, r, j] -> b = 4g + qi, s = 128t + 32h + r
            band[:, 128 * t + 32 * h + r_idx, :] = v.reshape(B, RW, 4)
    s_idx = np.arange(NT * SB)
    y = np.zeros((B, NT * SB, NT * SB, 4), dtype=np.float32)
    y[:, s_idx, s_idx, :] = band
    return y.reshape(B, NT * SB, NT * NI)
